# revision 5
# baseline (speedup 1.0000x reference)
"""Trainium2 Bass kernel v8 for nn_BERT_9070970929347.

Tiny BERT: B=4096, S=128, D=9, V=5, 4 single-head attention blocks, final
projection to 5 logits + log_softmax.  Data parallel over batch: 512
sequences per core on 8 cores.

v8 design (vs baseline):
  - A-trick: scores = X^T A X with A = sc*Wq_aug^T Wk_aug [10,10] folded on
    host; no Q/K projections on device.  scores mm: lhsT = Y_b = (A X)_b
    [10,128], rhs = X_b [10,128], both partition-base-0 (hard constraint:
    this stack rejects non-zero partition bases for matmul operands).
  - Y for the NEXT block rides the PV matmul: V2 = V_aug A_{i+1}^T folded
    into the V projection, so pv2 = E^T V2 normalizes into xnA = x' A^T,
    and the flip matmul emits [X_next^T | Y_next^T] stacked [80,128] in one
    shot -> one cheap lane-rich PSUM->SBUF copy into the resident xy buffer.
  - x/y in matmul-operand form ([10, g*512+128b+s]) produced by SBUF->SBUF
    DMAs (partition shift) on the idle SP queue, batched 16 groups per DMA.
  - 4 sequences per group (GE=4), 128 groups, zero padding.
  - PSUM mega-tiles with 4-group slots; exp/copies/normalize batched across
    slots (ACT fixed cost is ~293ns/instr -> batch everything).
  - log_softmax fused: strided exp, segmented reduce, Ln, broadcast subtract.
"""

import os
os.environ.setdefault("NEURON_RT_RESET_CORES", "1")
import numpy as np
import ml_dtypes
import sys

sys.path.insert(0, "/opt/trn_rl_repo")

import concourse.bass as bass
import concourse.mybir as mybir
from concourse import tile
from concourse.bass_utils import run_bass_kernel_spmd
from concourse.bass import AP

BF16 = ml_dtypes.bfloat16

B, S, D, V, NB = 4096, 128, 9, 5, 4
NCORES = 8
BPC = B // NCORES          # 512 sequences per core
GE = 4                     # sequences per group
NG = BPC // GE             # 128 groups per core
DA = D + 1                 # augmented dim (ones row)
SG = 4                     # groups per PSUM super-slot
NSG = NG // SG             # 32 psum-supers
DMAG = 16                  # groups per x10/y10 DMA batch
NDMA = NG // DMAG          # 8 DMA supers

_dt_bf16 = mybir.dt.bfloat16
_dt_f32 = mybir.dt.float32


def _pos_encoding(seq_len, dim):
    pos = np.arange(seq_len, dtype=np.float32)[:, None]
    d = np.arange(dim)[None, :]
    angle = pos / np.power(10000.0, (2.0 * (d // 2)) / dim).astype(np.float32)
    return np.where(d % 2 == 0, np.sin(angle), np.cos(angle)).astype(np.float32)


def _apv(sl, dims):
    """AP with explicit free-dim (stride, n) list; partition dim from sl."""
    lay = [list(sl.ap[0])] + [[s_, n_] for (s_, n_) in dims]
    return AP(sl.tensor, sl.offset, lay)


def build_nc():
    from concourse.bacc import Bacc
    nc = Bacc()

    xy0_in = nc.declare_dram_parameter("xy0", [2 * GE * DA, NG * 128], _dt_bf16,
                                       isOutput=False)
    wv2_in = nc.declare_dram_parameter("wv2", [NB, GE * DA, GE * 2 * DA], _dt_bf16,
                                       isOutput=False)
    wout_in = nc.declare_dram_parameter("wout", [GE * DA, GE * V], _dt_bf16,
                                        isOutput=False)
    ident_in = nc.declare_dram_parameter("ident", [128, 128], _dt_bf16,
                                         isOutput=False)
    out_ext = nc.declare_dram_parameter("out", [NG, 128, GE * V], _dt_f32,
                                        isOutput=True)

    XROWS = GE * DA            # 40
    XYROWS = 2 * GE * DA       # 80

    with tile.TileContext(nc) as tc:
        with (
            tc.tile_pool(name="consts", bufs=1) as cpool,
            tc.tile_pool(name="xybuf", bufs=1) as xypool,
            tc.tile_pool(name="xf", bufs=3) as xfp,
            tc.tile_pool(name="work", bufs=4) as wp,
            tc.tile_pool(name="fin", bufs=3) as fp,
            tc.tile_pool(name="psS", bufs=1, space="PSUM") as psS,
            tc.tile_pool(name="psV", bufs=1, space="PSUM") as psV,
            tc.tile_pool(name="psP", bufs=1, space="PSUM") as psP,
            tc.tile_pool(name="psF", bufs=1, space="PSUM") as psF,
            tc.tile_pool(name="psL", bufs=1, space="PSUM") as psL,
        ):
            # ---- constants ----
            wv2_sb = [cpool.tile([XROWS, GE * 2 * DA], _dt_bf16, tag=f"wv2{i}",
                                 name=f"wv2{i}") for i in range(NB)]
            wout_sb = cpool.tile([XROWS, GE * V], _dt_bf16, tag="wout")
            ident_sb = cpool.tile([128, 128], _dt_bf16, tag="ident")
            for i in range(NB):
                nc.sync.dma_start(out=wv2_sb[i][:], in_=wv2_in[i])
            nc.sync.dma_start(out=wout_sb[:], in_=wout_in[:])
            nc.sync.dma_start(out=ident_sb[:], in_=ident_in[:])

            # ---- resident x|y (stacked form), ping-pong generations ----
            xy = [xypool.tile([XYROWS, NG * 128], _dt_bf16, tag=f"xy{k}",
                              name=f"xy{k}") for k in range(2)]
            nc.sync.dma_start(out=xy[0][:], in_=xy0_in[:])

            # ---- PSUM megas ----
            scm = psS.tile([128, SG * 512], _dt_f32, tag="scm")      # 4 banks
            vm = psV.tile([128, SG * GE * 2 * DA], _dt_f32, tag="vm")   # 320
            pvm = psP.tile([128, SG * GE * 2 * DA], _dt_f32, tag="pvm")
            flm = psF.tile([XYROWS, SG * 128], _dt_f32, tag="flm")   # 1 bank
            lgm = psL.tile([128, SG * GE * V], _dt_f32, tag="lgm")   # 80 cols

            W2 = 2 * DA                # 20: V|V2 cols per seq
            GW = GE * W2               # 80: V|V2 cols per group

            # ================= attention blocks =================
            for i in range(NB):
                xin, xnext = xy[i % 2], xy[(i + 1) % 2]
                for jd in range(NDMA):
                    c0 = jd * DMAG * 128
                    x10 = xfp.tile([DA, DMAG * GE * 128], _dt_bf16, tag="x10")
                    y10 = xfp.tile([DA, DMAG * GE * 128], _dt_bf16, tag="y10")
                    # partition-shift DMAs: [10@10b, DMAG*128] -> x10 cols 128b+512g
                    for b in range(GE):
                        nc.sync.dma_start(
                            out=_apv(x10[:, 128 * b:], [(GE * 128, DMAG), (1, 128)]),
                            in_=xin[DA * b:DA * b + DA, c0:c0 + DMAG * 128]
                                .rearrange("d (g s) -> d g s", g=DMAG))
                        nc.sync.dma_start(
                            out=_apv(y10[:, 128 * b:], [(GE * 128, DMAG), (1, 128)]),
                            in_=xin[XROWS + DA * b:XROWS + DA * b + DA,
                                    c0:c0 + DMAG * 128]
                                .rearrange("d (g s) -> d g s", g=DMAG))
                    xg = xfp.tile([XROWS, DMAG * 128], _dt_bf16, tag="xg")
                    nc.sync.dma_start(out=xg[:],
                                      in_=xin[0:XROWS, c0:c0 + DMAG * 128])

                    for js in range(DMAG // SG):
                        j = jd * (DMAG // SG) + js       # psum-super index
                        g0 = j * SG
                        xq0 = js * SG * GE * 128         # x10 col of group g0
                        e4 = wp.tile([128, SG * 512], _dt_bf16, tag="e4")
                        v_sb = wp.tile([128, SG * GW], _dt_bf16, tag="v")
                        xn2 = wp.tile([128, SG * GW], _dt_bf16, tag="xn")
                        r4 = wp.tile([128, SG * GE], _dt_f32, tag="r")

                        # V|V2 projections (one mm per group)
                        for sl in range(SG):
                            gl = js * SG + sl
                            nc.tensor.matmul(
                                vm[:, sl * GW:(sl + 1) * GW],
                                lhsT=xg[:, gl * 128:(gl + 1) * 128],
                                rhs=wv2_sb[i][:], start=True, stop=True)
                        nc.vector.tensor_scalar_mul(v_sb[:], vm[:], 1.0)

                        # scores (one mm per sequence)
                        for sl in range(SG):
                            for b in range(GE):
                                q0 = xq0 + (sl * GE + b) * 128
                                nc.tensor.matmul(
                                    scm[:, (sl * GE + b) * 128:(sl * GE + b + 1) * 128],
                                    lhsT=y10[:, q0:q0 + 128],
                                    rhs=x10[:, q0:q0 + 128],
                                    start=True, stop=True, tile_position=(0, 0))
                        # exp in halves (2 groups each) for pipelining
                        nc.scalar.activation(e4[:], scm[:],
                                             mybir.ActivationFunctionType.Exp)

                        # PV: x'|x'A (one mm per sequence, 20 cols)
                        for sl in range(SG):
                            for b in range(GE):
                                nc.tensor.matmul(
                                    pvm[:, sl * GW + b * W2:sl * GW + (b + 1) * W2],
                                    lhsT=e4[:, (sl * GE + b) * 128:(sl * GE + b + 1) * 128],
                                    rhs=v_sb[:, sl * GW + b * W2:sl * GW + (b + 1) * W2],
                                    start=True, stop=True)
                        # denominators (V ones-column) -> reciprocal
                        nc.vector.reciprocal(
                            r4[:].rearrange("p (s b) -> p s b", s=SG),
                            _apv(pvm[:, D:], [(GW, SG), (W2, GE)]))
                        # normalize x' part and x'A part (broadcast multiply);
                        # xn2 per-group layout: [x(4*10) | y(4*10)] at 80*sl
                        nc.vector.tensor_tensor(
                            _apv(xn2[:, 0:], [(GW, SG), (DA, GE), (1, DA)]),
                            _apv(pvm[:, 0:], [(GW, SG), (W2, GE), (1, DA)]),
                            _apv(r4[:, 0:], [(GE, SG), (1, GE), (0, DA)]),
                            mybir.AluOpType.mult)
                        nc.vector.tensor_tensor(
                            _apv(xn2[:, GE * DA:], [(GW, SG), (DA, GE), (1, DA)]),
                            _apv(pvm[:, DA:], [(GW, SG), (W2, GE), (1, DA)]),
                            _apv(r4[:, 0:], [(GE, SG), (1, GE), (0, DA)]),
                            mybir.AluOpType.mult)

                        # flip: [x' | x'A]^T -> [X_next | Y_next] stacked
                        for sl in range(SG):
                            nc.tensor.matmul(
                                flm[:, sl * 128:(sl + 1) * 128],
                                lhsT=xn2[:, sl * GW:(sl + 1) * GW],
                                rhs=ident_sb[:], start=True, stop=True)
                        # one copy into resident xy
                        nc.vector.tensor_scalar_mul(
                            xnext[:, g0 * 128:(g0 + SG) * 128], flm[:], 1.0)

            # ================= final layer =================
            for j in range(NSG):
                g0 = j * SG
                xgf = fp.tile([XROWS, SG * 128], _dt_bf16, tag="xgf")
                nc.sync.dma_start(
                    out=xgf[:],
                    in_=xy[NB % 2][0:XROWS, g0 * 128:(g0 + SG) * 128])
                exb = fp.tile([128, SG * GE * V], _dt_f32, tag="exb")
                den = fp.tile([128, SG * GE], _dt_f32, tag="den")
                lnt = fp.tile([128, SG * GE], _dt_f32, tag="lnt")
                osb = fp.tile([128, SG * GE * V], _dt_f32, tag="osb")
                for sl in range(SG):
                    nc.tensor.matmul(
                        lgm[:, sl * GE * V:(sl + 1) * GE * V],
                        lhsT=xgf[:, sl * 128:(sl + 1) * 128],
                        rhs=wout_sb[:], start=True, stop=True)
                nc.scalar.activation(exb[:], lgm[:],
                                     mybir.ActivationFunctionType.Exp)
                nc.vector.tensor_reduce(
                    den[:], exb[:].rearrange("p (x v) -> p x v", v=V),
                    mybir.AxisListType.X, mybir.AluOpType.add)
                nc.scalar.activation(lnt[:], den[:],
                                     mybir.ActivationFunctionType.Ln)
                nc.vector.tensor_tensor(
                    osb[:].rearrange("p (x v) -> p x v", v=V),
                    lgm[:].rearrange("p (x v) -> p x v", v=V),
                    _apv(lnt[:, 0:], [(1, SG * GE), (0, V)]),
                    mybir.AluOpType.subtract)
                nc.sync.dma_start(
                    out=out_ext[g0:g0 + SG].rearrange("g p w -> p g w"),
                    in_=osb[:].rearrange("p (g w) -> p g w", g=SG))

    nc.compile()
    return nc
